# revision 5
# baseline (speedup 1.0000x reference)
"""ChebConv (order-4) GNN layer on 8 Trainium2 NeuronCores.

Reference computation (fp32):
    T0 = x, T1 = G x, Tk = 2 G T{k-1} - T{k-2}
    out = sum_k Tk @ W[k]          # [N, F] with N=10000, F=32

Strategy:
  * Rewrite in the power basis: y0 = x, yk = G y{k-1},
      out = sum_k yk @ Wp[k]  with
      Wp = [W0 - W2, W1 - 3 W3, 2 W2, 4 W3]   (exact modulo fp reassociation)
    so each hop is a bare matmul against G (no 2*/- epilogue).
  * Row-shard G over 8 cores (1280 padded rows each). The per-core lhsT
    tiles must hold G^T, so the host passes each core a contiguous
    GT_c = G[rows_c, :].T  of shape [10240, 1280] (pad N 10000->10240).
    Each 128-row j-chunk of GT_c is a contiguous 640 KB DMA.
  * Per hop, each core computes yk^T for its local rows: for each
    128-row j-chunk, matmul(lhsT=v[j-chunk] [128,32], rhs=GT tile
    [128,<=512]) accumulates [32,<=512] chunks of yk^T over all 80
    j-chunks (one open PSUM accumulation group per bank).
  * The Wp contraction happens on-chip from yk^T directly:
    matmul(lhsT=Wp_k [32,32], rhs=ykT chunk) then DVE-add into the
    transposed output accumulator; the k=0 term uses the host xT slice.
  * Between hops, yk^T is PE-transposed ([32,128] -> [128,32] blocks)
    into the natural m-chunk stage layout [128, 10*32], all-gathered
    (DRAM bounce, 160 KB per core), and reloaded into the
    [128, 80*32] j-chunk-tiled SBUF layout the next hop's lhsT wants.
    The last hop skips this entirely.
  * Output is returned transposed ([32, 1280] per core); the host
    concatenates, transposes and drops padding.
"""

import sys

if "/opt/trn_rl_repo" not in sys.path:
    sys.path.insert(0, "/opt/trn_rl_repo")

import numpy as np

N = 10000
F = 32
ORDER = 4
NCORES = 8
P = 128
NP = 10240  # padded node count: divisible by NCORES * P
RPC = NP // NCORES  # rows per core (1280)
JC = NP // P  # global 128-row chunks (80)
MC = RPC // P  # local 128-row chunks per core (10)

_CACHE = {}


def _build(np_total, ncores, dtype_g="float32"):
    from concourse import bacc, masks, mybir, tile

    rpc = np_total // ncores
    jc = np_total // P
    mc = rpc // P
    f32 = mybir.dt.float32
    fchunks = [(s, min(512, rpc - s)) for s in range(0, rpc, 512)]

    nc = bacc.Bacc(
        "TRN2", target_bir_lowering=False, debug=False, num_devices=ncores
    )
    gt = nc.dram_tensor("gt", [np_total, rpc], f32, kind="ExternalInput").ap()
    xtiles = nc.dram_tensor("xtiles", [P, jc * F], f32, kind="ExternalInput").ap()
    xt = nc.dram_tensor("xt", [F, rpc], f32, kind="ExternalInput").ap()
    wp = nc.dram_tensor("wp", [F, ORDER * F], f32, kind="ExternalInput").ap()
    out_t = nc.dram_tensor("outT", [F, rpc], f32, kind="ExternalOutput").ap()

    with tile.TileContext(nc) as tc:
        with (
            tc.tile_pool(name="const", bufs=1) as constp,
            tc.tile_pool(name="gtp", bufs=6) as gtp,
            tc.tile_pool(name="vp", bufs=2) as vp,
            tc.tile_pool(name="sb", bufs=2) as sb,
            tc.tile_pool(name="ps_hop", bufs=1, space="PSUM") as ps_hop,
            tc.tile_pool(name="ps_tp", bufs=2, space="PSUM") as ps_tp,
            tc.tile_pool(name="ps_w", bufs=2, space="PSUM") as ps_w,
            tc.tile_pool(name="dram", bufs=2, space="DRAM") as dram,
        ):
            ident = constp.tile([P, P], f32)
            masks.make_identity(nc, ident[:])
            w_sb = constp.tile([F, ORDER * F], f32)
            nc.scalar.dma_start(w_sb[:], wp)
            xt_sb = constp.tile([F, rpc], f32)
            nc.scalar.dma_start(xt_sb[:], xt)
            out_sb = constp.tile([F, rpc], f32)

            v_sb = vp.tile([P, jc * F], f32, tag="v")
            nc.scalar.dma_start(v_sb[:], xtiles)

            # k = 0 contribution: out^T = Wp_0^T @ x^T
            for s, l in fchunks:
                pw = ps_w.tile([F, l], f32, tag="pw")
                nc.tensor.matmul(
                    pw[:], lhsT=w_sb[:, 0:F], rhs=xt_sb[:, s : s + l],
                    start=True, stop=True,
                )
                nc.vector.tensor_copy(out_sb[:, s : s + l], pw[:])

            for k in range(1, ORDER):
                # hop: y_k^T = (G @ y_{k-1})^T for this core's rows,
                # accumulated over all j-chunks in PSUM
                hps = [
                    ps_hop.tile([F, l], f32, tag=f"hop{i}", name=f"hp{i}")
                    for i, (s, l) in enumerate(fchunks)
                ]
                for j in range(jc):
                    g = gtp.tile([P, rpc], f32, tag="gt")
                    nc.sync.dma_start(g[:], gt[j * P : (j + 1) * P, :])
                    for i, (s, l) in enumerate(fchunks):
                        nc.tensor.matmul(
                            hps[i][:],
                            lhsT=v_sb[:, j * F : (j + 1) * F],
                            rhs=g[:, s : s + l],
                            start=(j == 0),
                            stop=(j == jc - 1),
                        )
                y_t = sb.tile([F, rpc], f32, tag="yT")
                for i, (s, l) in enumerate(fchunks):
                    nc.vector.tensor_copy(y_t[:, s : s + l], hps[i][:])

                for s, l in fchunks:
                    pw = ps_w.tile([F, l], f32, tag="pw")
                    nc.tensor.matmul(
                        pw[:], lhsT=w_sb[:, k * F : (k + 1) * F],
                        rhs=y_t[:, s : s + l], start=True, stop=True,
                    )
                    nc.vector.tensor_add(
                        out_sb[:, s : s + l], out_sb[:, s : s + l], pw[:]
                    )

                if k < ORDER - 1:
                    # transpose y_k^T into natural m-chunk stage layout,
                    # then all-gather -> next hop's v_sb
                    stage = sb.tile([P, mc * F], f32, tag="stage")
                    for m in range(mc):
                        tp = ps_tp.tile([P, F], f32, tag="tp")
                        nc.tensor.transpose(
                            tp[:], y_t[:, m * P : (m + 1) * P], ident[0:F, 0:F]
                        )
                        nc.vector.tensor_copy(
                            stage[:, m * F : (m + 1) * F], tp[:]
                        )
                    cc_in = dram.tile([P, mc * F], f32, tag="ccin")
                    cc_out = dram.tile([ncores * P, mc * F], f32, tag="ccout")
                    nc.scalar.dma_start(cc_in[:], stage[:])
                    nc.gpsimd.collective_compute(
                        "AllGather",
                        mybir.AluOpType.bypass,
                        replica_groups=[list(range(ncores))],
                        ins=[cc_in.opt()],
                        outs=[cc_out.opt()],
                    )
                    v_sb = vp.tile([P, jc * F], f32, tag="v")
                    nc.scalar.dma_start(
                        v_sb[:].rearrange("p (c m) -> p c m", c=ncores),
                        cc_out[:].rearrange("(c p) m -> p c m", p=P),
                    )

            nc.scalar.dma_start(out_t, out_sb[:])

    nc.compile()
    return nc


def get_nc(np_total=NP, ncores=NCORES):
    key = (np_total, ncores)
    if key not in _CACHE:
        _CACHE[key] = _build(np_total, ncores)
    return _CACHE[key]


def prep_inputs(x, gso, weight, np_total=NP, ncores=NCORES):
    """Host-side shard prep. Returns in_maps for run_bass_kernel_spmd."""
    n = x.shape[0]
    rpc = np_total // ncores
    jc = np_total // P

    x = np.asarray(x, dtype=np.float32)
    gso = np.asarray(gso, dtype=np.float32)
    weight = np.asarray(weight, dtype=np.float32)

    wp = np.concatenate(
        [
            weight[0] - weight[2],
            weight[1] - 3.0 * weight[3],
            2.0 * weight[2],
            4.0 * weight[3],
        ],
        axis=1,
    ).astype(np.float32)  # [F, ORDER*F]

    xpad = np.zeros((np_total, F), dtype=np.float32)
    xpad[:n] = x
    gpad = np.zeros((np_total, np_total), dtype=np.float32)
    gpad[:n, :n] = gso

    # j-chunk-tiled x: xtiles[p, j*F+f] = xpad[j*P+p, f]
    xtiles = np.ascontiguousarray(
        xpad.reshape(jc, P, F).transpose(1, 0, 2).reshape(P, jc * F)
    )

    in_maps = []
    for c in range(ncores):
        rows = slice(c * rpc, (c + 1) * rpc)
        gt_c = np.ascontiguousarray(gpad[rows, :].T)  # [np_total, rpc]
        xt_c = np.ascontiguousarray(xpad[rows, :].T)  # [F, rpc]
        in_maps.append({"gt": gt_c, "xtiles": xtiles, "xt": xt_c, "wp": wp})
    return in_maps


def assemble_output(results, n=N, ncores=NCORES):
    out_t = np.concatenate([results[c]["outT"] for c in range(ncores)], axis=1)
    return np.ascontiguousarray(out_t.T[:n]).astype(np.float32)


def kernel(x, gso, weight):
    from concourse import bass_utils

    nc = get_nc()
    in_maps = prep_inputs(x, gso, weight)
    res = bass_utils.run_bass_kernel_spmd(
        nc, in_maps, core_ids=list(range(NCORES))
    )
    return assemble_output(res.results)


# revision 6
# speedup vs baseline: 1.0806x; 1.0806x over previous
"""ChebConv (order-4) GNN layer on 8 Trainium2 NeuronCores.

Reference computation (fp32):
    T0 = x, T1 = G x, Tk = 2 G T{k-1} - T{k-2}
    out = sum_k Tk @ W[k]          # [N, F] with N=10000, F=32

Strategy:
  * Rewrite in the power basis: y0 = x, yk = G y{k-1},
      out = sum_k yk @ Wp[k]  with
      Wp = [W0 - W2, W1 - 3 W3, 2 W2, 4 W3]   (exact modulo fp reassociation)
    so each hop is a bare matmul against G (no 2*/- epilogue).
  * Row-shard G over 8 cores (1280 padded rows each). The per-core lhsT
    tiles must hold G^T, so the host passes each core a contiguous
    transposed slice (pad N 10000 -> 10240).
  * fp32 matmuls on the TRN2 PE run in LOW_HIGH mode: 2 passes, each
    streaming the fp32 rhs at half rate (4x bf16 cost). Instead we do a
    software hi/lo split: G = G_hi + G_lo and v = v_hi + v_lo (bf16
    pairs) and compute G_hi v_hi + G_lo v_hi + G_hi v_lo with fp32 PSUM
    accumulation -- 3 full-rate bf16 passes, same DRAM bytes as fp32,
    ~7e-6 relative error (vs 3e-3 for plain bf16). G_hi/G_lo rows are
    interleaved in one [NP, 2, rpc] array so each 128-row j-chunk is a
    single contiguous 640 KB DMA.
  * Per hop, each core computes yk^T for its local rows: for each
    128-row j-chunk, matmuls (lhsT=v_{hi,lo}[j-chunk] [128,32] bf16,
    rhs=G^T_{hi,lo} tile [128,<=512] bf16) accumulate [32,<=512] chunks
    of yk^T over all 80 j-chunks (one open accumulation group per bank).
  * The Wp contraction happens on-chip from yk^T in full fp32:
    matmul(lhsT=Wp_k [32,32], rhs=ykT chunk), DVE-add into the
    transposed output accumulator; the k=0 term uses the host xT slice.
  * Between hops, ykT is PE-transposed ([32,128] -> [128,32] blocks)
    into natural m-chunk stage layout, split into bf16 hi/lo halves,
    all-gathered in one collective (DRAM bounce, 160 KB per core), and
    reloaded into the [128, 80*32] j-chunk-tiled v_hi/v_lo SBUF layout
    the next hop's lhsT wants. The last hop skips this entirely.
  * Output is returned transposed ([32, 1280] per core); the host
    concatenates, transposes and drops padding.
"""

import sys

if "/opt/trn_rl_repo" not in sys.path:
    sys.path.insert(0, "/opt/trn_rl_repo")

import numpy as np

N = 10000
F = 32
ORDER = 4
NCORES = 8
P = 128
NP = 10240  # padded node count: divisible by NCORES * P
RPC = NP // NCORES  # rows per core (1280)
JC = NP // P  # global 128-row chunks (80)
MC = RPC // P  # local 128-row chunks per core (10)

_CACHE = {}


def _build(np_total, ncores):
    from concourse import bacc, masks, mybir, tile

    rpc = np_total // ncores
    jc = np_total // P
    mc = rpc // P
    f32 = mybir.dt.float32
    bf16 = mybir.dt.bfloat16
    fchunks = [(s, min(512, rpc - s)) for s in range(0, rpc, 512)]
    sg = mc * F  # stage columns per half (hi or lo)

    nc = bacc.Bacc(
        "TRN2", target_bir_lowering=False, debug=False, num_devices=ncores
    )
    ghl = nc.dram_tensor("ghl", [np_total, 2 * rpc], bf16, kind="ExternalInput").ap()
    xthl = nc.dram_tensor("xthl", [P, 2 * jc * F], bf16, kind="ExternalInput").ap()
    xt = nc.dram_tensor("xt", [F, rpc], f32, kind="ExternalInput").ap()
    wp = nc.dram_tensor("wp", [F, ORDER * F], f32, kind="ExternalInput").ap()
    out_t = nc.dram_tensor("outT", [F, rpc], f32, kind="ExternalOutput").ap()

    with tile.TileContext(nc) as tc:
        with (
            tc.tile_pool(name="const", bufs=1) as constp,
            tc.tile_pool(name="gtp", bufs=6) as gtp,
            tc.tile_pool(name="vp", bufs=2) as vp,
            tc.tile_pool(name="sb", bufs=2) as sb,
            tc.tile_pool(name="ps_hop", bufs=1, space="PSUM") as ps_hop,
            tc.tile_pool(name="ps_tp", bufs=2, space="PSUM") as ps_tp,
            tc.tile_pool(name="ps_w", bufs=2, space="PSUM") as ps_w,
            tc.tile_pool(name="dram", bufs=2, space="DRAM") as dram,
        ):
            ident = constp.tile([P, P], f32)
            masks.make_identity(nc, ident[:])
            w_sb = constp.tile([F, ORDER * F], f32)
            nc.scalar.dma_start(w_sb[:], wp)
            xt_sb = constp.tile([F, rpc], f32)
            nc.scalar.dma_start(xt_sb[:], xt)
            out_sb = constp.tile([F, rpc], f32)

            # v holds y_{k-1} as a bf16 hi/lo pair in j-chunk-tiled layout
            v_sb = vp.tile([P, 2 * jc * F], bf16, tag="v")
            nc.scalar.dma_start(v_sb[:], xthl)

            def v_hi(j):
                return v_sb[:, j * F : (j + 1) * F]

            def v_lo(j):
                return v_sb[:, (jc + j) * F : (jc + j + 1) * F]

            # k = 0 contribution: out^T = Wp_0^T @ x^T (pure fp32)
            for s, l in fchunks:
                pw = ps_w.tile([F, l], f32, tag="pw")
                nc.tensor.matmul(
                    pw[:], lhsT=w_sb[:, 0:F], rhs=xt_sb[:, s : s + l],
                    start=True, stop=True,
                )
                nc.vector.tensor_copy(out_sb[:, s : s + l], pw[:])

            for k in range(1, ORDER):
                # hop: y_k^T = (G @ y_{k-1})^T via 3 bf16 hi/lo passes
                hps = [
                    ps_hop.tile([F, l], f32, tag=f"hop{i}", name=f"hp{i}")
                    for i, (s, l) in enumerate(fchunks)
                ]
                nfc = len(fchunks)
                for j in range(jc):
                    g = gtp.tile([P, 2 * rpc], bf16, tag="gt")
                    nc.sync.dma_start(g[:], ghl[j * P : (j + 1) * P, :])
                    for i, (s, l) in enumerate(fchunks):
                        gh = g[:, s : s + l]
                        gl = g[:, rpc + s : rpc + s + l]
                        for t, (lhs, rhs) in enumerate(
                            ((v_hi(j), gh), (v_lo(j), gh), (v_hi(j), gl))
                        ):
                            nc.tensor.matmul(
                                hps[i][:], lhsT=lhs, rhs=rhs,
                                start=(j == 0 and t == 0),
                                stop=(j == jc - 1 and t == 2),
                            )
                y_t = sb.tile([F, rpc], f32, tag="yT")
                for i, (s, l) in enumerate(fchunks):
                    nc.vector.tensor_copy(y_t[:, s : s + l], hps[i][:])

                for s, l in fchunks:
                    pw = ps_w.tile([F, l], f32, tag="pw")
                    nc.tensor.matmul(
                        pw[:], lhsT=w_sb[:, k * F : (k + 1) * F],
                        rhs=y_t[:, s : s + l], start=True, stop=True,
                    )
                    nc.vector.tensor_add(
                        out_sb[:, s : s + l], out_sb[:, s : s + l], pw[:]
                    )

                if k < ORDER - 1:
                    # transpose y_k^T into natural m-chunk layout, split
                    # bf16 hi/lo, all-gather, reload as next v
                    stage = sb.tile([P, 2 * sg], bf16, tag="stage")
                    for m in range(mc):
                        tp = ps_tp.tile([P, F], f32, tag="tp")
                        nc.tensor.transpose(
                            tp[:], y_t[:, m * P : (m + 1) * P], ident[0:F, 0:F]
                        )
                        hi = stage[:, m * F : (m + 1) * F]
                        lo = stage[:, sg + m * F : sg + (m + 1) * F]
                        nc.vector.tensor_copy(hi, tp[:])
                        nc.vector.tensor_sub(lo, tp[:], hi)
                    cc_in = dram.tile([P, 2 * sg], bf16, tag="ccin")
                    cc_out = dram.tile([ncores * P, 2 * sg], bf16, tag="ccout")
                    nc.scalar.dma_start(cc_in[:], stage[:])
                    nc.gpsimd.collective_compute(
                        "AllGather",
                        mybir.AluOpType.bypass,
                        replica_groups=[list(range(ncores))],
                        ins=[cc_in.opt()],
                        outs=[cc_out.opt()],
                    )
                    v_sb = vp.tile([P, 2 * jc * F], bf16, tag="v")
                    # hi half: v[:, 0:jc*F], lo half: v[:, jc*F:2*jc*F]
                    nc.scalar.dma_start(
                        v_sb[:, 0 : jc * F].rearrange("p (c m) -> p c m", c=ncores),
                        cc_out[:, 0:sg].rearrange("(c p) m -> p c m", p=P),
                    )
                    nc.scalar.dma_start(
                        v_sb[:, jc * F : 2 * jc * F].rearrange(
                            "p (c m) -> p c m", c=ncores
                        ),
                        cc_out[:, sg : 2 * sg].rearrange("(c p) m -> p c m", p=P),
                    )

            nc.scalar.dma_start(out_t, out_sb[:])

    nc.compile()
    return nc


def get_nc(np_total=NP, ncores=NCORES):
    key = (np_total, ncores)
    if key not in _CACHE:
        _CACHE[key] = _build(np_total, ncores)
    return _CACHE[key]


def _bf16_pair(a):
    import ml_dtypes

    hi = a.astype(ml_dtypes.bfloat16)
    lo = (a - hi.astype(np.float32)).astype(ml_dtypes.bfloat16)
    return hi, lo


def prep_inputs(x, gso, weight, np_total=NP, ncores=NCORES):
    """Host-side shard prep. Returns in_maps for run_bass_kernel_spmd."""
    n = x.shape[0]
    rpc = np_total // ncores
    jc = np_total // P

    x = np.asarray(x, dtype=np.float32)
    gso = np.asarray(gso, dtype=np.float32)
    weight = np.asarray(weight, dtype=np.float32)

    wp = np.concatenate(
        [
            weight[0] - weight[2],
            weight[1] - 3.0 * weight[3],
            2.0 * weight[2],
            4.0 * weight[3],
        ],
        axis=1,
    ).astype(np.float32)  # [F, ORDER*F]

    xpad = np.zeros((np_total, F), dtype=np.float32)
    xpad[:n] = x
    gpad = np.zeros((np_total, np_total), dtype=np.float32)
    gpad[:n, :n] = gso
    g_hi, g_lo = _bf16_pair(gpad)

    # j-chunk-tiled x as bf16 hi/lo pair: [(P, jc*F) hi | (P, jc*F) lo]
    x_hi, x_lo = _bf16_pair(xpad)

    def tile_x(a):
        return a.reshape(jc, P, F).transpose(1, 0, 2).reshape(P, jc * F)

    xthl = np.ascontiguousarray(
        np.concatenate([tile_x(x_hi), tile_x(x_lo)], axis=1)
    )

    in_maps = []
    for c in range(ncores):
        rows = slice(c * rpc, (c + 1) * rpc)
        # interleave per-row: ghl[j, 0, :] = G_hi^T row, ghl[j, 1, :] = G_lo^T
        ghl_c = np.ascontiguousarray(
            np.stack([g_hi[rows, :].T, g_lo[rows, :].T], axis=1).reshape(
                np_total, 2 * rpc
            )
        )
        xt_c = np.ascontiguousarray(xpad[rows, :].T)  # [F, rpc] fp32
        in_maps.append({"ghl": ghl_c, "xthl": xthl, "xt": xt_c, "wp": wp})
    return in_maps


def assemble_output(results, n=N, ncores=NCORES):
    out_t = np.concatenate([results[c]["outT"] for c in range(ncores)], axis=1)
    return np.ascontiguousarray(out_t.T[:n]).astype(np.float32)


def kernel(x, gso, weight):
    from concourse import bass_utils

    nc = get_nc()
    in_maps = prep_inputs(x, gso, weight)
    res = bass_utils.run_bass_kernel_spmd(
        nc, in_maps, core_ids=list(range(NCORES))
    )
    return assemble_output(res.results)
